# revision 2
# baseline (speedup 1.0000x reference)
"""Trainium2 Bass kernel for nn_Decoder (LayerNorm-LSTM decoder).

Data-parallel over batch: B=2048 sharded as 256 rows/core across 8 cores.
Per core: 2 chunks of 128 partitions; T=28 recurrent steps, all SBUF-resident.

Key reformulation (verified vs reference in numpy):
  z = zx_t + LN(h@W)*rn_g + rn_b + bias, with zx_t = LN(x_t@K)*kn_g + kn_b.
  Since DIN=4, zx_t is rank-4 + row stats, so each step's pre-gate tensor is ONE
  PSUM accumulation:  zq = h @ (W*rn_g)  +  [x~*sig; -mu; sig] @ [K_c*kn_g; rn_g; colconst]
  and gates = func(s * zq) using ACT's per-partition scale (s = rsqrt(var+eps)).
  r-stats without materializing r: mu = h@rowmean(W) (tiny matmul),
  sumsq = rowsum((h@G) * h) with G = W@W^T (PE matmul + DVE tensor_tensor_reduce).
  rsqrt via bit-trick + Newton on DVE (no ACT table switches: every ACT func used
  -- Sigmoid/Tanh/Square/Identity/Copy/Relu -- lives in the sigmoid_and_others set).
"""

import sys

sys.path.insert(0, "/opt/trn_rl_repo")

import numpy as np

import concourse.bass as bass
import concourse.bacc as bacc
import concourse.tile as tile
from concourse import mybir
from concourse.bass_utils import run_bass_kernel_spmd
from concourse.masks import make_identity

F32 = mybir.dt.float32
U32 = mybir.dt.uint32
I32 = mybir.dt.int32
AF = mybir.ActivationFunctionType
OP = mybir.AluOpType
X_AX = mybir.AxisListType.X

N_CORES = 8
B, T, DIN, U = 2048, 28, 4, 512
FU = 4 * U            # 2048
BL = B // N_CORES     # 256 rows per core
NM = BL // 128        # 2 partition chunks per core
EPS = np.float32(1e-3)

GATE_FUNCS = [AF.Sigmoid, AF.Sigmoid, AF.Tanh, AF.Sigmoid]  # i, f, g, o


def _emit_rsqrt(nc, tiny, t_ap, n, iters):
    """y ~= rsqrt(t) elementwise on a [128, n] f32 AP via bit-trick + Newton."""
    sh = tiny.tile([128, n], U32, tag="tiny_u")
    nc.vector.tensor_scalar(sh[:], t_ap.bitcast(U32), 1, None,
                            OP.logical_shift_right)
    y_i = tiny.tile([128, n], I32, tag="tiny_u")
    # 0x5f3759df - (i>>1) == (i>>1)*(-1) + 0x5f3759df ; operands < 2^31 so the
    # int32 view is safe, and seed rounding is irrelevant (Newton fixes it).
    nc.vector.tensor_scalar(y_i[:], sh[:].bitcast(I32), -1, 0x5F3759DF,
                            OP.mult, OP.add)
    y = y_i[:].bitcast(F32)
    for _ in range(iters):
        p = tiny.tile([128, n], F32, tag="tiny")
        nc.vector.tensor_mul(p[:], t_ap, y)
        p2 = tiny.tile([128, n], F32, tag="tiny")
        nc.vector.tensor_mul(p2[:], p[:], y)
        w = tiny.tile([128, n], F32, tag="tiny")
        nc.vector.tensor_scalar(w[:], p2[:], -0.5, 1.5, OP.mult, OP.add)
        y2 = tiny.tile([128, n], F32, tag="tiny")
        nc.vector.tensor_mul(y2[:], y, w[:])
        y = y2[:]
    return y


def _build(weights, t_steps=T):
    (K, W, bias_v, kn_g, kn_b, rn_g, rn_b, sn_g, sn_b, Wd, bd) = weights

    # ---- host-side weight-only preprocessing ----
    k_mean = K.mean(axis=1).astype(np.float32)                    # [4]
    K_c = (K - k_mean[:, None]).astype(np.float32)                # [4, FU]
    K_x = (K_c * kn_g[None, :]).astype(np.float32)
    colconst = (kn_b + bias_v + rn_b).astype(np.float32)          # [FU]
    W_eff = (W * rn_g[None, :]).astype(np.float32)                # [U, FU]
    w_mean = W.mean(axis=1).astype(np.float32)                    # [U]
    G = (W @ W.T).astype(np.float32)                              # [U, U]
    M4aug = np.concatenate([K @ K.T, k_mean[:, None]], 1).astype(np.float32)  # [4,5]
    Ktil = np.concatenate(
        [K_x, rn_g[None, :], colconst[None, :]], axis=0).astype(np.float32)   # [6,FU]
    sn_trivial = bool(np.all(sn_g == 1.0) and np.all(sn_b == 0.0))

    nc = bacc.Bacc("TRN2", target_bir_lowering=False, debug=False,
                   num_devices=N_CORES)

    x_ext = nc.declare_dram_parameter("x", [BL, T, DIN], F32, isOutput=False)
    xT_ext = nc.declare_dram_parameter("xT", [DIN, T, BL], F32, isOutput=False)
    h0_ext = nc.declare_dram_parameter("h0", [BL, U], F32, isOutput=False)
    c0_ext = nc.declare_dram_parameter("c0", [BL, U], F32, isOutput=False)
    out_ext = nc.declare_dram_parameter("out", [BL, t_steps, DIN], F32, isOutput=True)

    W_d = nc.inline_tensor(np.ascontiguousarray(
        W_eff.reshape(4, 128, FU)), name="W_eff")
    G_d = nc.inline_tensor(np.ascontiguousarray(
        G.reshape(4, 128, U)), name="G")
    wm_d = nc.inline_tensor(np.ascontiguousarray(
        w_mean.reshape(4, 128).T), name="wmean")                  # [128, 4]
    Kt_d = nc.inline_tensor(Ktil, name="Ktil")                    # [6, FU]
    M4_d = nc.inline_tensor(M4aug, name="M4aug")                  # [4, 5]
    Wd_d = nc.inline_tensor(np.ascontiguousarray(
        Wd.reshape(4, 128, 4)), name="Wd")
    bd_d = nc.inline_tensor(np.ascontiguousarray(
        np.tile(bd[None, :], (128, 1)).astype(np.float32)), name="bd_rep")
    if not sn_trivial:
        sng_d = nc.inline_tensor(np.ascontiguousarray(
            np.tile(sn_g[None, :], (128, 1)).astype(np.float32)), name="sng")
        snb_d = nc.inline_tensor(np.ascontiguousarray(
            np.tile(sn_b[None, :], (128, 1)).astype(np.float32)), name="snb")

    with tile.TileContext(nc) as tc:
        ctx_pools = []

        def pool(**kw):
            p = tc.tile_pool(**kw)
            ctx_pools.append(p)
            return p.__enter__()

        const = pool(name="const", bufs=1)
        state = pool(name="state", bufs=1)
        cpool = pool(name="cstate", bufs=2)
        hpool = pool(name="hstate", bufs=2)
        gpool = pool(name="gates", bufs=2)
        tmp = pool(name="tmp", bufs=6)
        scr = pool(name="scr", bufs=2)
        tiny = pool(name="tiny", bufs=28)
        stgp = pool(name="stg", bufs=3)
        lhp = pool(name="lhsT", bufs=3)
        zqp = pool(name="zq", bufs=5, space="PSUM")
        aux = pool(name="aux", bufs=3, space="PSUM")

        # ---------------- constants into SBUF ----------------
        W_sb = []
        G_sb = []
        for uc in range(4):
            w_t = const.tile([128, FU], F32, tag=f"W{uc}")
            nc.sync.dma_start(w_t[:], W_d[uc])
            W_sb.append(w_t)
            g_t = const.tile([128, U], F32, tag=f"G{uc}")
            nc.sync.dma_start(g_t[:], G_d[uc])
            G_sb.append(g_t)
        wm_sb = const.tile([128, 4], F32, tag="wm")
        nc.sync.dma_start(wm_sb[:], wm_d[:, :])
        Kt_sb = const.tile([6, FU], F32, tag="Kt")
        nc.sync.dma_start(Kt_sb[:], Kt_d[:, :])
        M4_sb = const.tile([4, 5], F32, tag="M4")
        nc.sync.dma_start(M4_sb[:], M4_d[:, :])
        Wd_sb = const.tile([128, 4, 4], F32, tag="Wd")
        nc.sync.dma_start(Wd_sb[:], Wd_d[:, :, :].rearrange("u p d -> p u d"))
        bd_sb = const.tile([128, 4], F32, tag="bd")
        nc.sync.dma_start(bd_sb[:], bd_d[:, :])
        if not sn_trivial:
            sng_sb = const.tile([128, U], F32, tag="sng")
            nc.sync.dma_start(sng_sb[:], sng_d[:, :])
            snb_sb = const.tile([128, U], F32, tag="snb")
            nc.sync.dma_start(snb_sb[:], snb_d[:, :])
        ident = const.tile([128, 128], F32, tag="ident")
        make_identity(nc, ident[:])

        # anchor the ACT table set (sigmoid_and_others holds every func we use)
        dummy = tiny.tile([128, 1], F32, tag="tiny")
        nc.vector.memset(dummy[:], 0.0)
        nc.scalar.activation(dummy[:], dummy[:], AF.Sigmoid)

        # ---------------- load x / h0 / c0 ----------------
        x_sb = state.tile([128, NM, T * DIN], F32, tag="x")
        nc.sync.dma_start(
            x_sb[:], x_ext[:, :, :].rearrange("(m p) t d -> p m (t d)", p=128))
        xuT_sb = state.tile([DIN, T, BL], F32, tag="xuT")
        nc.sync.dma_start(xuT_sb[:], xT_ext[:, :, :])
        h_prev = hpool.tile([128, NM, U], F32, tag="h")
        nc.sync.dma_start(
            h_prev[:], h0_ext[:, :].rearrange("(m p) u -> p m u", p=128))
        c_prev = cpool.tile([128, NM, U], F32, tag="c")
        nc.sync.dma_start(
            c_prev[:], c0_ext[:, :].rearrange("(m p) u -> p m u", p=128))

        hT_sb = state.tile([128, 4, BL], F32, tag="hT")
        out_sb = state.tile([128, NM, t_steps * DIN], F32, tag="out")
        sx_sb = state.tile([128, NM, T], F32, tag="sx")

        def mcols(m):
            return slice(m * 128, (m + 1) * 128)

        # h0 -> hT
        for m in range(NM):
            htp = aux.tile([128, 512], F32, tag="aux")
            for uc in range(4):
                nc.tensor.transpose(
                    htp[:, uc * 128:(uc + 1) * 128],
                    h_prev[:, m, uc * 128:(uc + 1) * 128], ident[:])
            for uc in range(4):
                eng = nc.scalar if uc % 2 == 0 else nc.vector
                if eng is nc.scalar:
                    nc.scalar.copy(hT_sb[:, uc, mcols(m)],
                                   htp[:, uc * 128:(uc + 1) * 128])
                else:
                    nc.vector.tensor_copy(hT_sb[:, uc, mcols(m)],
                                          htp[:, uc * 128:(uc + 1) * 128])

        # ---------------- x precompute: s_x = rsqrt(var_x + eps) ----------------
        for m in range(NM):
            # q = x @ [K K^T | k_mean] per t  -> [128, 28*5]
            qx_ps = aux.tile([128, 512], F32, tag="aux")
            for t in range(T):
                nc.tensor.matmul(qx_ps[:, t * 5:(t + 1) * 5],
                                 xuT_sb[0:4, t, mcols(m)], M4_sb[:, :],
                                 start=True, stop=True)
            qx_sb = tmp.tile([128, 144], F32, tag="qx")
            nc.scalar.copy(qx_sb[:, 0:T * 5], qx_ps[:, 0:T * 5])
            qv = qx_sb[:, 0:T * 5].rearrange("p (t e) -> p t e", e=5)
            xv = x_sb[:, m, :].rearrange("p (t d) -> p t d", d=DIN)
            pq = tmp.tile([128, T * DIN], F32, tag="pq")
            nc.vector.tensor_mul(
                pq[:].rearrange("p (t d) -> p t d", d=DIN), qv[:, :, 0:4], xv)
            ssx = tiny.tile([128, T], F32, tag="tinyT")
            nc.vector.reduce_sum(ssx[:], pq[:].rearrange(
                "p (t d) -> p t d", d=DIN), axis=X_AX)
            mux = qv[:, :, 4]                                    # [128, 28]
            msq = tiny.tile([128, T], F32, tag="tinyT")
            nc.vector.tensor_mul(msq[:], mux, mux)
            t1m = tiny.tile([128, T], F32, tag="tinyT")
            nc.vector.tensor_scalar(t1m[:], msq[:], -1.0, float(EPS),
                                    OP.mult, OP.add)
            am = tiny.tile([128, T], F32, tag="tinyT")
            nc.vector.tensor_scalar(am[:], ssx[:], 1.0 / FU, None, OP.mult)
            tx = tiny.tile([128, T], F32, tag="tinyT")
            nc.vector.tensor_add(tx[:], am[:], t1m[:])
            sx = _emit_rsqrt(nc, tiny, tx[:], T, 3)              # [128, 28]
            nc.vector.tensor_copy(sx_sb[:, m, :], sx)

        # ---------------- recurrent steps ----------------
        for t in range(t_steps):
            s_list = [None] * NM
            lh_list = [None] * NM
            zq_list = [None] * NM
            # stats matmuls for both chunks first (decouple from r-matmuls)
            stats_ps = []
            for m in range(NM):
                mu_ps = aux.tile([128, 512], F32, tag="aux")
                for uc in range(4):
                    nc.tensor.matmul(mu_ps[:, 0:1], hT_sb[:, uc, mcols(m)],
                                     wm_sb[:, uc:uc + 1],
                                     start=(uc == 0), stop=(uc == 3))
                u_ps = aux.tile([128, 512], F32, tag="aux")
                for uc in range(4):
                    nc.tensor.matmul(u_ps[:, :], hT_sb[:, uc, mcols(m)],
                                     G_sb[uc][:, :],
                                     start=(uc == 0), stop=(uc == 3))
                stats_ps.append((mu_ps, u_ps))

            for m in range(NM):
                mu_ps, u_ps = stats_ps[m]
                # r matmuls: zq[n] accumulates h@W_eff (k-outer for LDW reuse)
                zqs = [zqp.tile([128, 512], F32, tag="zq", name=f"zq{n}")
                       for n in range(4)]
                zq_list[m] = zqs
                for uc in range(4):
                    for n in range(4):
                        nc.tensor.matmul(zqs[n][:, :],
                                         hT_sb[:, uc, mcols(m)],
                                         W_sb[uc][:, n * 512:(n + 1) * 512],
                                         start=(uc == 0), stop=False)
                # z-LN stats on DVE: sumsq(r) = rowsum((h@G) * h)
                scr_t = scr.tile([128, 512], F32, tag="scr")
                nc.vector.tensor_mul(scr_t[:], u_ps[:, :], h_prev[:, m, :])
                ssr = tiny.tile([128, 1], F32, tag="tiny")
                nc.vector.reduce_sum(ssr[:], scr_t[:], axis=X_AX)
                mum = tiny.tile([128, 1], F32, tag="tiny")
                nc.scalar.copy(mum[:], mu_ps[:, 0:1])
                msq = tiny.tile([128, 1], F32, tag="tiny")
                nc.vector.tensor_mul(msq[:], mum[:], mum[:])
                t1m = tiny.tile([128, 1], F32, tag="tiny")
                nc.vector.tensor_scalar(t1m[:], msq[:], -1.0, float(EPS),
                                        OP.mult, OP.add)
                am = tiny.tile([128, 1], F32, tag="tiny")
                nc.vector.tensor_scalar(am[:], ssr[:], 1.0 / FU, None, OP.mult)
                tz = tiny.tile([128, 1], F32, tag="tiny")
                nc.vector.tensor_add(tz[:], am[:], t1m[:])
                s_m = _emit_rsqrt(nc, tiny, tz[:], 1, 2)         # [128,1]
                s_list[m] = s_m
                sg = tiny.tile([128, 1], F32, tag="tiny")
                nc.vector.tensor_mul(sg[:], tz[:], s_m)          # sqrt(var+eps)
                # build augmented lhsT rows: [x_T*(s_x*sig)(4); -mu(1); sig(1)]
                sxs = tiny.tile([128, 1], F32, tag="tiny")
                nc.vector.tensor_mul(sxs[:], sg[:], sx_sb[:, m, t:t + 1])
                stg = stgp.tile([128, 8], F32, tag="stg")
                nc.vector.tensor_copy(stg[:, 0:4], sxs[:].broadcast_to(
                    [128, 4]))
                nc.vector.tensor_scalar(stg[:, 4:5], mum[:], -1.0, None,
                                        OP.mult)
                nc.vector.tensor_copy(stg[:, 5:6], sg[:])
                smT = aux.tile([128, 512], F32, tag="aux")
                nc.tensor.transpose(smT[0:6, 0:128], stg[:, 0:6], ident[:])
                lh = lhp.tile([6, 128], F32, tag="lh")
                lh_list[m] = lh
                nc.scalar.copy(lh[0:6, :], smT[0:6, 0:128])
                nc.vector.tensor_mul(
                    lh[0:4, :], xuT_sb[0:4, t, mcols(m)], smT[0:4, 0:128])
                for n in range(4):
                    nc.tensor.matmul(zqs[n][:, :], lh[:, :],
                                     Kt_sb[:, n * 512:(n + 1) * 512],
                                     start=False, stop=True)

            gates = gpool.tile([128, NM, FU], F32, tag="g")
            for m in range(NM):
                for n in range(4):
                    nc.scalar.activation(gates[:, m, n * 512:(n + 1) * 512],
                                         zq_list[m][n][:, :], GATE_FUNCS[n],
                                         scale=s_list[m])

            # ---- c update + c-LN + h ----
            c_cur = cpool.tile([128, NM, U], F32, tag="c")
            h_cur = hpool.tile([128, NM, U], F32, tag="h")
            for m in range(NM):
                t1 = tmp.tile([128, U], F32, tag="t1")
                nc.vector.tensor_mul(t1[:], gates[:, m, 512:1024],
                                     c_prev[:, m, :])
                t2 = tmp.tile([128, U], F32, tag="t2")
                nc.vector.tensor_mul(t2[:], gates[:, m, 0:512],
                                     gates[:, m, 1024:1536])
                cc = tmp.tile([128, U], F32, tag="cc")
                nc.vector.tensor_add(cc[:], t1[:], t2[:])
                st6 = tiny.tile([128, 6], F32, tag="tiny")
                nc.vector.bn_stats(st6[:], cc[:])
                mv = tiny.tile([128, 2], F32, tag="tiny")
                nc.vector.bn_aggr(mv[:], st6[:])
                tcm = tiny.tile([128, 1], F32, tag="tiny")
                nc.vector.tensor_scalar(tcm[:], mv[:, 1:2], float(EPS), None,
                                        OP.add)
                sc = _emit_rsqrt(nc, tiny, tcm[:], 1, 2)
                nmsc = tiny.tile([128, 1], F32, tag="tiny")
                nc.vector.tensor_scalar(nmsc[:], mv[:, 0:1], sc, -1.0,
                                        OP.mult, OP.mult)
                # normalized c (state) on ACT; tanh with fused affine
                if sn_trivial:
                    nc.scalar.activation(c_cur[:, m, :], cc[:], AF.Identity,
                                         bias=nmsc[:], scale=sc)
                    th = tmp.tile([128, U], F32, tag="th")
                    nc.scalar.activation(th[:], cc[:], AF.Tanh,
                                         bias=nmsc[:], scale=sc)
                else:
                    cn0 = tmp.tile([128, U], F32, tag="cn0")
                    nc.scalar.activation(cn0[:], cc[:], AF.Identity,
                                         bias=nmsc[:], scale=sc)
                    cn1 = tmp.tile([128, U], F32, tag="cn1")
                    nc.vector.tensor_mul(cn1[:], cn0[:], sng_sb[:, :])
                    nc.vector.tensor_add(c_cur[:, m, :], cn1[:], snb_sb[:, :])
                    th = tmp.tile([128, U], F32, tag="th")
                    nc.scalar.activation(th[:], c_cur[:, m, :], AF.Tanh)
                nc.vector.tensor_mul(h_cur[:, m, :], gates[:, m, 1536:2048],
                                     th[:])
                # h -> hT for next step / output matmul
                htp = aux.tile([128, 512], F32, tag="aux")
                for uc in range(4):
                    nc.tensor.transpose(htp[:, uc * 128:(uc + 1) * 128],
                                        h_cur[:, m, uc * 128:(uc + 1) * 128],
                                        ident[:])
                for uc in range(4):
                    if uc % 2 == 0:
                        nc.scalar.copy(hT_sb[:, uc, mcols(m)],
                                       htp[:, uc * 128:(uc + 1) * 128])
                    else:
                        nc.vector.tensor_copy(hT_sb[:, uc, mcols(m)],
                                              htp[:, uc * 128:(uc + 1) * 128])
                # out_t = relu(h @ Wd + bd)
                op_ps = aux.tile([128, 512], F32, tag="aux")
                for uc in range(4):
                    nc.tensor.matmul(op_ps[:, 0:4], hT_sb[:, uc, mcols(m)],
                                     Wd_sb[:, uc, :],
                                     start=(uc == 0), stop=(uc == 3))
                ob = tiny.tile([128, 4], F32, tag="tiny")
                nc.vector.tensor_add(ob[:], op_ps[:, 0:4], bd_sb[:, :])
                nc.vector.tensor_scalar(out_sb[:, m, t * 4:(t + 1) * 4],
                                        ob[:], 0.0, None, OP.max)
            c_prev = c_cur
            h_prev = h_cur

        nc.sync.dma_start(
            out_ext[:, :, :].rearrange("(m p) t d -> p m (t d)", p=128),
            out_sb[:])

        for p in reversed(ctx_pools):
            p.__exit__(None, None, None)

    nc.compile()
    return nc


_NC_CACHE = {}


def _get_nc(weights):
    key = tuple(hash(w.tobytes()) for w in weights)
    if key not in _NC_CACHE:
        _NC_CACHE.clear()
        _NC_CACHE[key] = _build(weights)
    return _NC_CACHE[key]


def _make_in_maps(inputs):
    f32 = lambda a: np.ascontiguousarray(np.asarray(a, dtype=np.float32))
    x = f32(inputs["x"])
    h0 = f32(inputs["h0"])
    c0 = f32(inputs["c0"])
    return [
        {
            "x": np.ascontiguousarray(x[i * BL:(i + 1) * BL]),
            "xT": np.ascontiguousarray(
                x[i * BL:(i + 1) * BL].transpose(2, 1, 0)),
            "h0": np.ascontiguousarray(h0[i * BL:(i + 1) * BL]),
            "c0": np.ascontiguousarray(c0[i * BL:(i + 1) * BL]),
        }
        for i in range(N_CORES)
    ]


def kernel(**inputs):
    f32 = lambda a: np.ascontiguousarray(np.asarray(a, dtype=np.float32))
    weights = (
        f32(inputs["kernel"]), f32(inputs["rec_kernel"]), f32(inputs["bias"]),
        f32(inputs["kn_g"]), f32(inputs["kn_b"]), f32(inputs["rn_g"]),
        f32(inputs["rn_b"]), f32(inputs["sn_g"]), f32(inputs["sn_b"]),
        f32(inputs["Wd"]), f32(inputs["bd"]),
    )
    nc = _get_nc(weights)
    in_maps = _make_in_maps(inputs)
    res = run_bass_kernel_spmd(nc, in_maps, core_ids=list(range(N_CORES)))
    out = np.concatenate([res.results[i]["out"] for i in range(N_CORES)],
                         axis=0)
    return out.astype(np.float32)


if __name__ == "__main__":
    np.random.seed(0)
    pass



# revision 10
# speedup vs baseline: 1.6192x; 1.6192x over previous
"""Trainium2 Bass kernel for nn_Decoder (LayerNorm-LSTM decoder).

Data-parallel over batch: B=2048 sharded as 256 rows/core across 8 cores.
Per core: 2 chunks of 128 partitions; T=28 recurrent steps, all SBUF-resident.

Reformulation (verified vs reference):
  z = zx_t + LN(h@W)*rn_g + rn_b + bias, with zx_t = LN(x_t@K)*kn_g + kn_b.
  Each step's pre-gate tensor is ONE PSUM accumulation:
    zq = h @ (W*rn_g) + aug @ Ktil,  gates = act(s * zq [+ beta])
  with s = rsqrt(var_r+eps) applied via ACT's per-partition scale.
  r-stats without materializing r: mu = h@rowmean(W), sumsq = rowsum((h@G)*h)
  with G = W@W^T (PE matmul + DVE mul/reduce).
  When rn_g is uniform and kn_b+bias+rn_b==0 (the shipped weights), -mu folds
  into the ACT bias (beta = -rn_g*s*mu) and aug shrinks to the 4 x-rows.
  rsqrt via bit-trick + Newton on DVE.
  Big matmuls run as float32r (fp32 data, 4x PE throughput at free>=256).
"""

import sys

sys.path.insert(0, "/opt/trn_rl_repo")

import numpy as np

import concourse.bass as bass
import concourse.bacc as bacc
import concourse.tile as tile
from concourse import mybir
from concourse.bass_utils import run_bass_kernel_spmd
from concourse.masks import make_identity

F32 = mybir.dt.float32
F32R = mybir.dt.float32r
U32 = mybir.dt.uint32
I32 = mybir.dt.int32
AF = mybir.ActivationFunctionType
OP = mybir.AluOpType
X_AX = mybir.AxisListType.X

N_CORES = 8
B, T, DIN, U = 2048, 28, 4, 512
FU = 4 * U            # 2048
BL = B // N_CORES     # 256 rows per core
NM = BL // 128        # 2 partition chunks per core
EPS = np.float32(1e-3)

GATE_FUNCS = [AF.Sigmoid, AF.Sigmoid, AF.Tanh, AF.Sigmoid]  # i, f, g, o


def _emit_rsqrt(nc, tiny, t_ap, n, iters):
    """y ~= rsqrt(t) elementwise on a [128, n] f32 AP via bit-trick + Newton."""
    sh = tiny.tile([128, n], U32, tag="tiny_u")
    nc.vector.tensor_scalar(sh[:], t_ap.bitcast(U32), 1, None,
                            OP.logical_shift_right)
    y_i = tiny.tile([128, n], I32, tag="tiny_u")
    nc.vector.tensor_scalar(y_i[:], sh[:].bitcast(I32), -1, 0x5F3759DF,
                            OP.mult, OP.add)
    y = y_i[:].bitcast(F32)
    for _ in range(iters):
        p = tiny.tile([128, n], F32, tag="tiny")
        nc.vector.tensor_mul(p[:], t_ap, y)
        p2 = tiny.tile([128, n], F32, tag="tiny")
        nc.vector.tensor_mul(p2[:], p[:], y)
        w = tiny.tile([128, n], F32, tag="tiny")
        nc.vector.tensor_scalar(w[:], p2[:], -0.5, 1.5, OP.mult, OP.add)
        y2 = tiny.tile([128, n], F32, tag="tiny")
        nc.vector.tensor_mul(y2[:], y, w[:])
        y = y2[:]
    return y


def _build(weights, t_steps=T):
    (K, W, bias_v, kn_g, kn_b, rn_g, rn_b, sn_g, sn_b, Wd, bd) = weights

    # ---- host-side weight-only preprocessing ----
    k_mean = K.mean(axis=1).astype(np.float32)                    # [4]
    K_c = (K - k_mean[:, None]).astype(np.float32)                # [4, FU]
    K_x = (K_c * kn_g[None, :]).astype(np.float32)
    colconst = (kn_b + bias_v + rn_b).astype(np.float32)          # [FU]
    W_eff = (W * rn_g[None, :]).astype(np.float32)                # [U, FU]
    w_mean = W.mean(axis=1).astype(np.float32)                    # [U]
    G = (W @ W.T).astype(np.float32)                              # [U, U]
    M4aug = np.concatenate([K @ K.T, k_mean[:, None]], 1).astype(np.float32)
    sn_trivial = bool(np.all(sn_g == 1.0) and np.all(sn_b == 0.0))
    # -mu*rn_g + colconst folds into the ACT bias iff rn_g is uniform and
    # colconst == 0 (the shipped decoder: all gains 1, all biases 0).
    aug_trivial = bool(np.all(rn_g == rn_g[0]) and np.all(colconst == 0.0))
    rng0 = float(rn_g[0])
    bd_trivial = bool(np.all(bd == 0.0))
    if aug_trivial:
        Ktil = K_x                                                # [4, FU]
    else:
        Ktil = np.concatenate(
            [K_x, rn_g[None, :], colconst[None, :]], axis=0).astype(np.float32)
    NAUG = Ktil.shape[0]

    nc = bacc.Bacc("TRN2", target_bir_lowering=False, debug=False,
                   num_devices=N_CORES)

    x_ext = nc.declare_dram_parameter("x", [BL, T, DIN], F32, isOutput=False)
    xT_ext = nc.declare_dram_parameter("xT", [DIN, T, BL], F32, isOutput=False)
    h0_ext = nc.declare_dram_parameter("h0", [BL, U], F32, isOutput=False)
    c0_ext = nc.declare_dram_parameter("c0", [BL, U], F32, isOutput=False)
    out_ext = nc.declare_dram_parameter("out", [BL, t_steps, DIN], F32,
                                        isOutput=True)

    W_d = nc.inline_tensor(np.ascontiguousarray(
        W_eff.reshape(4, 128, FU)), name="W_eff")
    G_d = nc.inline_tensor(np.ascontiguousarray(
        G.reshape(4, 128, U)), name="G")
    wm_d = nc.inline_tensor(np.ascontiguousarray(
        w_mean.reshape(4, 128).T), name="wmean")                  # [128, 4]
    Kt_d = nc.inline_tensor(np.ascontiguousarray(Ktil), name="Ktil")
    M4_d = nc.inline_tensor(M4aug, name="M4aug")                  # [4, 5]
    Wd_d = nc.inline_tensor(np.ascontiguousarray(
        Wd.reshape(4, 128, 4)), name="Wd")
    bd_d = nc.inline_tensor(np.ascontiguousarray(
        np.tile(bd[None, :], (128, 1)).astype(np.float32)), name="bd_rep")
    if not sn_trivial:
        sng_d = nc.inline_tensor(np.ascontiguousarray(
            np.tile(sn_g[None, :], (128, 1)).astype(np.float32)), name="sng")
        snb_d = nc.inline_tensor(np.ascontiguousarray(
            np.tile(sn_b[None, :], (128, 1)).astype(np.float32)), name="snb")

    with tile.TileContext(nc) as tc:
        ctx_pools = []

        def pool(**kw):
            p = tc.tile_pool(**kw)
            ctx_pools.append(p)
            return p.__enter__()

        const = pool(name="const", bufs=1)
        state = pool(name="state", bufs=1)
        cpool = pool(name="cstate", bufs=2)
        hpool = pool(name="hstate", bufs=2)
        gpool = pool(name="gates", bufs=2)
        tmp = pool(name="tmp", bufs=6)
        scr = pool(name="scr", bufs=2)
        tiny = pool(name="tiny", bufs=28)
        stgp = pool(name="stg", bufs=3)
        lhp = pool(name="lhsT", bufs=3)
        zqp = pool(name="zq", bufs=5, space="PSUM")
        aux = pool(name="aux", bufs=3, space="PSUM")

        # ---------------- constants into SBUF ----------------
        W_sb = []
        G_sb = []
        for uc in range(4):
            w_t = const.tile([128, FU], F32R, tag=f"W{uc}")
            nc.sync.dma_start(w_t[:].bitcast(F32), W_d[uc])
            W_sb.append(w_t)
            g_t = const.tile([128, U], F32R, tag=f"G{uc}")
            nc.sync.dma_start(g_t[:].bitcast(F32), G_d[uc])
            G_sb.append(g_t)
        wm_sb = const.tile([128, 4], F32, tag="wm")
        nc.sync.dma_start(wm_sb[:], wm_d[:, :])
        Kt_sb = const.tile([NAUG, FU], F32R, tag="Kt")
        nc.sync.dma_start(Kt_sb[:].bitcast(F32), Kt_d[:, :])
        M4_sb = const.tile([4, 5], F32, tag="M4")
        nc.sync.dma_start(M4_sb[:], M4_d[:, :])
        Wd_sb = const.tile([128, 4, 4], F32, tag="Wd")
        nc.sync.dma_start(Wd_sb[:], Wd_d[:, :, :].rearrange("u p d -> p u d"))
        bd_sb = const.tile([128, 4], F32, tag="bd")
        nc.sync.dma_start(bd_sb[:], bd_d[:, :])
        if not sn_trivial:
            sng_sb = const.tile([128, U], F32, tag="sng")
            nc.sync.dma_start(sng_sb[:], sng_d[:, :])
            snb_sb = const.tile([128, U], F32, tag="snb")
            nc.sync.dma_start(snb_sb[:], snb_d[:, :])
        ident = const.tile([128, 128], F32, tag="ident")
        make_identity(nc, ident[:])

        # anchor the ACT table set (sigmoid_and_others holds every func used)
        dummy = tiny.tile([128, 1], F32, tag="tiny")
        nc.vector.memset(dummy[:], 0.0)
        nc.scalar.activation(dummy[:], dummy[:], AF.Sigmoid)

        # ---------------- load x / h0 / c0 ----------------
        x_sb = state.tile([128, NM, T * DIN], F32, tag="x")
        nc.sync.dma_start(
            x_sb[:], x_ext[:, :, :].rearrange("(m p) t d -> p m (t d)", p=128))
        xuT_sb = state.tile([DIN, T, BL], F32, tag="xuT")
        nc.sync.dma_start(xuT_sb[:], xT_ext[:, :, :])
        h_prev = hpool.tile([128, NM, U], F32, tag="h")
        nc.sync.dma_start(
            h_prev[:], h0_ext[:, :].rearrange("(m p) u -> p m u", p=128))
        c_prev = cpool.tile([128, NM, U], F32, tag="c")
        nc.sync.dma_start(
            c_prev[:], c0_ext[:, :].rearrange("(m p) u -> p m u", p=128))

        hT_sb = state.tile([128, 4, BL], F32R, tag="hT")
        out_sb = state.tile([128, NM, t_steps * DIN], F32, tag="out")
        sx_sb = state.tile([128, NM, T], F32, tag="sx")

        def mcols(m):
            return slice(m * 128, (m + 1) * 128)

        def hTf(uc, m):
            return hT_sb[:, uc, mcols(m)].bitcast(F32)

        # h0 -> hT
        for m in range(NM):
            htp = aux.tile([128, 512], F32, tag="aux")
            for uc in range(4):
                nc.tensor.transpose(
                    htp[:, uc * 128:(uc + 1) * 128],
                    h_prev[:, m, uc * 128:(uc + 1) * 128], ident[:])
            for uc in range(4):
                if uc % 2 == 0:
                    nc.scalar.copy(hT_sb[:, uc, mcols(m)],
                                   htp[:, uc * 128:(uc + 1) * 128])
                else:
                    nc.vector.tensor_copy(hT_sb[:, uc, mcols(m)],
                                          htp[:, uc * 128:(uc + 1) * 128])

        # ------------- x precompute: s_x = rsqrt(var_x + eps) -------------
        for m in range(NM):
            qx_ps = aux.tile([128, 512], F32, tag="aux")
            for t in range(T):
                nc.tensor.matmul(qx_ps[:, t * 5:(t + 1) * 5],
                                 xuT_sb[0:4, t, mcols(m)], M4_sb[:, :],
                                 start=True, stop=True)
            qx_sb = tmp.tile([128, 144], F32, tag="qx")
            nc.scalar.copy(qx_sb[:, 0:T * 5], qx_ps[:, 0:T * 5])
            qv = qx_sb[:, 0:T * 5].rearrange("p (t e) -> p t e", e=5)
            xv = x_sb[:, m, :].rearrange("p (t d) -> p t d", d=DIN)
            pq = tmp.tile([128, T * DIN], F32, tag="pq")
            nc.vector.tensor_mul(
                pq[:].rearrange("p (t d) -> p t d", d=DIN), qv[:, :, 0:4], xv)
            ssx = tiny.tile([128, T], F32, tag="tinyT")
            nc.vector.reduce_sum(ssx[:], pq[:].rearrange(
                "p (t d) -> p t d", d=DIN), axis=X_AX)
            mux = qv[:, :, 4]                                    # [128, 28]
            msq = tiny.tile([128, T], F32, tag="tinyT")
            nc.vector.tensor_mul(msq[:], mux, mux)
            t1m = tiny.tile([128, T], F32, tag="tinyT")
            nc.vector.tensor_scalar(t1m[:], msq[:], -1.0, float(EPS),
                                    OP.mult, OP.add)
            am = tiny.tile([128, T], F32, tag="tinyT")
            nc.vector.tensor_scalar(am[:], ssx[:], 1.0 / FU, None, OP.mult)
            tx = tiny.tile([128, T], F32, tag="tinyT")
            nc.vector.tensor_add(tx[:], am[:], t1m[:])
            sx = _emit_rsqrt(nc, tiny, tx[:], T, 3)              # [128, 28]
            nc.vector.tensor_copy(sx_sb[:, m, :], sx)

        # ---------------- recurrent steps ----------------
        for t in range(t_steps):
            s_list = [None] * NM
            b_list = [None] * NM
            zq_list = [None] * NM
            stats_ps = []
            for m in range(NM):
                mu_ps = aux.tile([128, 512], F32, tag="aux")
                for uc in range(4):
                    nc.tensor.matmul(mu_ps[:, 0:1], hTf(uc, m),
                                     wm_sb[:, uc:uc + 1],
                                     start=(uc == 0), stop=(uc == 3))
                u_ps = aux.tile([128, 512], F32, tag="aux")
                for uc in range(4):
                    nc.tensor.matmul(u_ps[:, :], hT_sb[:, uc, mcols(m)],
                                     G_sb[uc][:, :],
                                     start=(uc == 0), stop=(uc == 3))
                stats_ps.append((mu_ps, u_ps))

            for m in range(NM):
                mu_ps, u_ps = stats_ps[m]
                zqs = [zqp.tile([128, 512], F32, tag="zq", name=f"zq{n}")
                       for n in range(4)]
                zq_list[m] = zqs
                for uc in range(4):
                    for n in range(4):
                        nc.tensor.matmul(zqs[n][:, :],
                                         hT_sb[:, uc, mcols(m)],
                                         W_sb[uc][:, n * 512:(n + 1) * 512],
                                         start=(uc == 0), stop=False)
                # z-LN stats on DVE: sumsq(r) = rowsum((h@G) * h)
                scr_t = scr.tile([128, 512], F32, tag="scr")
                nc.vector.tensor_mul(scr_t[:], u_ps[:, :], h_prev[:, m, :])
                ssr = tiny.tile([128, 1], F32, tag="tiny")
                nc.vector.reduce_sum(ssr[:], scr_t[:], axis=X_AX)
                mum = tiny.tile([128, 1], F32, tag="tiny")
                nc.scalar.copy(mum[:], mu_ps[:, 0:1])
                msq = tiny.tile([128, 1], F32, tag="tiny")
                nc.vector.tensor_mul(msq[:], mum[:], mum[:])
                t1m = tiny.tile([128, 1], F32, tag="tiny")
                nc.vector.tensor_scalar(t1m[:], msq[:], -1.0, float(EPS),
                                        OP.mult, OP.add)
                am = tiny.tile([128, 1], F32, tag="tiny")
                nc.vector.tensor_scalar(am[:], ssr[:], 1.0 / FU, None, OP.mult)
                tz = tiny.tile([128, 1], F32, tag="tiny")
                nc.vector.tensor_add(tz[:], am[:], t1m[:])
                s_m = _emit_rsqrt(nc, tiny, tz[:], 1, 1)         # [128,1]
                s_list[m] = s_m
                sg = tiny.tile([128, 1], F32, tag="tiny")
                nc.vector.tensor_mul(sg[:], tz[:], s_m)          # sqrt(var+eps)
                sxs = tiny.tile([128, 1], F32, tag="tiny")
                nc.vector.tensor_mul(sxs[:], sg[:], sx_sb[:, m, t:t + 1])
                stg = stgp.tile([128, 8], F32, tag="stg")
                nc.vector.tensor_copy(stg[:, 0:4], sxs[:].broadcast_to(
                    [128, 4]))
                if aug_trivial:
                    beta = tiny.tile([128, 1], F32, tag="tiny")
                    nc.vector.tensor_scalar(beta[:], mum[:], s_m, -rng0,
                                            OP.mult, OP.mult)
                    b_list[m] = beta
                else:
                    nc.vector.tensor_scalar(stg[:, 4:5], mum[:], -1.0, None,
                                            OP.mult)
                    nc.vector.tensor_copy(stg[:, 5:6], sg[:])
                smT = aux.tile([128, 512], F32, tag="aux")
                nc.tensor.transpose(smT[0:NAUG, 0:128], stg[:, 0:NAUG],
                                    ident[:])
                lh = lhp.tile([NAUG, 128], F32R, tag="lh")
                if not aug_trivial:
                    nc.scalar.copy(lh[0:NAUG, :], smT[0:NAUG, 0:128])
                nc.vector.tensor_mul(
                    lh[0:4, :], xuT_sb[0:4, t, mcols(m)], smT[0:4, 0:128])
                for n in range(4):
                    nc.tensor.matmul(zqs[n][:, :], lh[:, :],
                                     Kt_sb[:, n * 512:(n + 1) * 512],
                                     start=False, stop=True)

            gates = gpool.tile([128, NM, FU], F32, tag="g")
            for m in range(NM):
                for n in range(4):
                    if aug_trivial:
                        nc.scalar.activation(
                            gates[:, m, n * 512:(n + 1) * 512],
                            zq_list[m][n][:, :], GATE_FUNCS[n],
                            bias=b_list[m], scale=s_list[m])
                    else:
                        nc.scalar.activation(
                            gates[:, m, n * 512:(n + 1) * 512],
                            zq_list[m][n][:, :], GATE_FUNCS[n],
                            scale=s_list[m])

            # ---- c update + c-LN + h ----
            c_cur = cpool.tile([128, NM, U], F32, tag="c")
            h_cur = hpool.tile([128, NM, U], F32, tag="h")
            for m in range(NM):
                t1 = tmp.tile([128, U], F32, tag="t1")
                nc.vector.tensor_mul(t1[:], gates[:, m, 512:1024],
                                     c_prev[:, m, :])
                t2 = tmp.tile([128, U], F32, tag="t2")
                nc.vector.tensor_mul(t2[:], gates[:, m, 0:512],
                                     gates[:, m, 1024:1536])
                cc = tmp.tile([128, U], F32, tag="cc")
                nc.vector.tensor_add(cc[:], t1[:], t2[:])
                st6 = tiny.tile([128, 6], F32, tag="tiny")
                nc.vector.bn_stats(st6[:], cc[:])
                mv = tiny.tile([128, 2], F32, tag="tiny")
                nc.vector.bn_aggr(mv[:], st6[:])
                tcm = tiny.tile([128, 1], F32, tag="tiny")
                nc.vector.tensor_scalar(tcm[:], mv[:, 1:2], float(EPS), None,
                                        OP.add)
                sc = _emit_rsqrt(nc, tiny, tcm[:], 1, 1)
                nmsc = tiny.tile([128, 1], F32, tag="tiny")
                nc.vector.tensor_scalar(nmsc[:], mv[:, 0:1], sc, -1.0,
                                        OP.mult, OP.mult)
                # normalized c (state) on ACT; tanh with fused affine
                if sn_trivial:
                    nc.scalar.activation(c_cur[:, m, :], cc[:], AF.Identity,
                                         bias=nmsc[:], scale=sc)
                    th = tmp.tile([128, U], F32, tag="th")
                    nc.scalar.activation(th[:], cc[:], AF.Tanh,
                                         bias=nmsc[:], scale=sc)
                else:
                    cn0 = tmp.tile([128, U], F32, tag="cn0")
                    nc.scalar.activation(cn0[:], cc[:], AF.Identity,
                                         bias=nmsc[:], scale=sc)
                    cn1 = tmp.tile([128, U], F32, tag="cn1")
                    nc.vector.tensor_mul(cn1[:], cn0[:], sng_sb[:, :])
                    nc.vector.tensor_add(c_cur[:, m, :], cn1[:], snb_sb[:, :])
                    th = tmp.tile([128, U], F32, tag="th")
                    nc.scalar.activation(th[:], c_cur[:, m, :], AF.Tanh)
                nc.vector.tensor_mul(h_cur[:, m, :], gates[:, m, 1536:2048],
                                     th[:])
                # h -> hT for next step / output matmul
                htp = aux.tile([128, 512], F32, tag="aux")
                for uc in range(4):
                    nc.tensor.transpose(htp[:, uc * 128:(uc + 1) * 128],
                                        h_cur[:, m, uc * 128:(uc + 1) * 128],
                                        ident[:])
                for uc in range(4):
                    if uc % 2 == 0:
                        nc.scalar.copy(hT_sb[:, uc, mcols(m)],
                                       htp[:, uc * 128:(uc + 1) * 128])
                    else:
                        nc.vector.tensor_copy(hT_sb[:, uc, mcols(m)],
                                              htp[:, uc * 128:(uc + 1) * 128])
                # out_t = relu(h @ Wd + bd)
                op_ps = aux.tile([128, 512], F32, tag="aux")
                for uc in range(4):
                    nc.tensor.matmul(op_ps[:, 0:4], hTf(uc, m),
                                     Wd_sb[:, uc, :],
                                     start=(uc == 0), stop=(uc == 3))
                if bd_trivial:
                    nc.vector.tensor_scalar(out_sb[:, m, t * 4:(t + 1) * 4],
                                            op_ps[:, 0:4], 0.0, None, OP.max)
                else:
                    ob = tiny.tile([128, 4], F32, tag="tiny")
                    nc.vector.tensor_add(ob[:], op_ps[:, 0:4], bd_sb[:, :])
                    nc.vector.tensor_scalar(out_sb[:, m, t * 4:(t + 1) * 4],
                                            ob[:], 0.0, None, OP.max)
            c_prev = c_cur
            h_prev = h_cur

        nc.sync.dma_start(
            out_ext[:, :, :].rearrange("(m p) t d -> p m (t d)", p=128),
            out_sb[:])

        for p in reversed(ctx_pools):
            p.__exit__(None, None, None)

    nc.compile()
    return nc


_NC_CACHE = {}


def _get_nc(weights):
    key = tuple(hash(w.tobytes()) for w in weights)
    if key not in _NC_CACHE:
        _NC_CACHE.clear()
        _NC_CACHE[key] = _build(weights)
    return _NC_CACHE[key]


def _make_in_maps(inputs):
    f32 = lambda a: np.ascontiguousarray(np.asarray(a, dtype=np.float32))
    x = f32(inputs["x"])
    h0 = f32(inputs["h0"])
    c0 = f32(inputs["c0"])
    return [
        {
            "x": np.ascontiguousarray(x[i * BL:(i + 1) * BL]),
            "xT": np.ascontiguousarray(
                x[i * BL:(i + 1) * BL].transpose(2, 1, 0)),
            "h0": np.ascontiguousarray(h0[i * BL:(i + 1) * BL]),
            "c0": np.ascontiguousarray(c0[i * BL:(i + 1) * BL]),
        }
        for i in range(N_CORES)
    ]


def kernel(**inputs):
    f32 = lambda a: np.ascontiguousarray(np.asarray(a, dtype=np.float32))
    weights = (
        f32(inputs["kernel"]), f32(inputs["rec_kernel"]), f32(inputs["bias"]),
        f32(inputs["kn_g"]), f32(inputs["kn_b"]), f32(inputs["rn_g"]),
        f32(inputs["rn_b"]), f32(inputs["sn_g"]), f32(inputs["sn_b"]),
        f32(inputs["Wd"]), f32(inputs["bd"]),
    )
    nc = _get_nc(weights)
    in_maps = _make_in_maps(inputs)
    res = run_bass_kernel_spmd(nc, in_maps, core_ids=list(range(N_CORES)))
    out = np.concatenate([res.results[i]["out"] for i in range(N_CORES)],
                         axis=0)
    return out.astype(np.float32)


if __name__ == "__main__":
    np.random.seed(0)
    pass


# revision 11
# speedup vs baseline: 1.8260x; 1.1277x over previous
"""Trainium2 Bass kernel for nn_Decoder (LayerNorm-LSTM decoder).

Data-parallel over batch: B=2048 sharded as 256 rows/core across 8 cores.
Per core: 2 chunks of 128 partitions; T=28 recurrent steps, all SBUF-resident.

Reformulation (verified vs reference):
  z = zx_t + LN(h@W)*rn_g + rn_b + bias, with zx_t = LN(x_t@K)*kn_g + kn_b.
  Each step's pre-gate tensor is ONE PSUM accumulation:
    zq = h @ (W*rn_g) + aug @ Ktil,  gates = act(s * zq [+ beta])
  with s = rsqrt(var_r+eps) applied via ACT's per-partition scale.
  r-stats without materializing r: mu = h@rowmean(W), sumsq = rowsum((h@G)*h)
  with G = W@W^T (PE matmul + DVE mul/reduce).
  When rn_g is uniform and kn_b+bias+rn_b==0 (the shipped weights), -mu folds
  into the ACT bias (beta = -rn_g*s*mu) and aug shrinks to the 4 x-rows.
  rsqrt via bit-trick + Newton on DVE.
  Big matmuls run as float32r (fp32 data, 4x PE throughput at free>=256).
"""

import sys

sys.path.insert(0, "/opt/trn_rl_repo")

import ml_dtypes
import numpy as np

import concourse.bass as bass
import concourse.bacc as bacc
import concourse.tile as tile
from concourse import mybir
from concourse.bass_utils import run_bass_kernel_spmd
from concourse.masks import make_identity

F32 = mybir.dt.float32
F32R = mybir.dt.float32r
BF16 = mybir.dt.bfloat16
NPBF16 = np.dtype(ml_dtypes.bfloat16)
U32 = mybir.dt.uint32
I32 = mybir.dt.int32
AF = mybir.ActivationFunctionType
OP = mybir.AluOpType
X_AX = mybir.AxisListType.X

N_CORES = 8
B, T, DIN, U = 2048, 28, 4, 512
FU = 4 * U            # 2048
BL = B // N_CORES     # 256 rows per core
NM = BL // 128        # 2 partition chunks per core
EPS = np.float32(1e-3)

GATE_FUNCS = [AF.Sigmoid, AF.Sigmoid, AF.Tanh, AF.Sigmoid]  # i, f, g, o


def _emit_rsqrt(nc, tiny, t_ap, n, iters):
    """y ~= rsqrt(t) elementwise on a [128, n] f32 AP via bit-trick + Newton."""
    sh = tiny.tile([128, n], U32, tag="tiny_u")
    nc.vector.tensor_scalar(sh[:], t_ap.bitcast(U32), 1, None,
                            OP.logical_shift_right)
    y_i = tiny.tile([128, n], I32, tag="tiny_u")
    nc.vector.tensor_scalar(y_i[:], sh[:].bitcast(I32), -1, 0x5F3759DF,
                            OP.mult, OP.add)
    y = y_i[:].bitcast(F32)
    for _ in range(iters):
        p = tiny.tile([128, n], F32, tag="tiny")
        nc.vector.tensor_mul(p[:], t_ap, y)
        p2 = tiny.tile([128, n], F32, tag="tiny")
        nc.vector.tensor_mul(p2[:], p[:], y)
        w = tiny.tile([128, n], F32, tag="tiny")
        nc.vector.tensor_scalar(w[:], p2[:], -0.5, 1.5, OP.mult, OP.add)
        y2 = tiny.tile([128, n], F32, tag="tiny")
        nc.vector.tensor_mul(y2[:], y, w[:])
        y = y2[:]
    return y


def _build(weights, t_steps=T):
    (K, W, bias_v, kn_g, kn_b, rn_g, rn_b, sn_g, sn_b, Wd, bd) = weights

    # ---- host-side weight-only preprocessing ----
    k_mean = K.mean(axis=1).astype(np.float32)                    # [4]
    K_c = (K - k_mean[:, None]).astype(np.float32)                # [4, FU]
    K_x = (K_c * kn_g[None, :]).astype(np.float32)
    colconst = (kn_b + bias_v + rn_b).astype(np.float32)          # [FU]
    W_eff = (W * rn_g[None, :]).astype(np.float32)                # [U, FU]
    w_mean = W.mean(axis=1).astype(np.float32)                    # [U]
    G = (W @ W.T).astype(np.float32)                              # [U, U]
    M4aug = np.concatenate([K @ K.T, k_mean[:, None]], 1).astype(np.float32)
    sn_trivial = bool(np.all(sn_g == 1.0) and np.all(sn_b == 0.0))
    # -mu*rn_g + colconst folds into the ACT bias iff rn_g is uniform and
    # colconst == 0 (the shipped decoder: all gains 1, all biases 0).
    aug_trivial = bool(np.all(rn_g == rn_g[0]) and np.all(colconst == 0.0))
    rng0 = float(rn_g[0])
    bd_trivial = bool(np.all(bd == 0.0))
    if aug_trivial:
        Ktil = K_x                                                # [4, FU]
    else:
        Ktil = np.concatenate(
            [K_x, rn_g[None, :], colconst[None, :]], axis=0).astype(np.float32)
    NAUG = Ktil.shape[0]

    nc = bacc.Bacc("TRN2", target_bir_lowering=False, debug=False,
                   num_devices=N_CORES)

    x_ext = nc.declare_dram_parameter("x", [BL, T, DIN], F32, isOutput=False)
    xT_ext = nc.declare_dram_parameter("xT", [DIN, T, BL], F32, isOutput=False)
    h0_ext = nc.declare_dram_parameter("h0", [BL, U], F32, isOutput=False)
    c0_ext = nc.declare_dram_parameter("c0", [BL, U], F32, isOutput=False)
    out_ext = nc.declare_dram_parameter("out", [BL, t_steps, DIN], F32,
                                        isOutput=True)

    W_d = nc.inline_tensor(np.ascontiguousarray(
        W_eff.reshape(4, 128, FU).astype(NPBF16)), name="W_eff")
    G_d = nc.inline_tensor(np.ascontiguousarray(
        G.reshape(4, 128, U).astype(NPBF16)), name="G")
    wm_d = nc.inline_tensor(np.ascontiguousarray(
        w_mean.reshape(4, 128).T.astype(NPBF16)), name="wmean")                  # [128, 4]
    Kt_d = nc.inline_tensor(np.ascontiguousarray(
        Ktil.astype(NPBF16)), name="Ktil")
    M4_d = nc.inline_tensor(M4aug, name="M4aug")                  # [4, 5]
    Wd_d = nc.inline_tensor(np.ascontiguousarray(
        Wd.reshape(4, 128, 4).astype(NPBF16)), name="Wd")
    bd_d = nc.inline_tensor(np.ascontiguousarray(
        np.tile(bd[None, :], (128, 1)).astype(np.float32)), name="bd_rep")
    if not sn_trivial:
        sng_d = nc.inline_tensor(np.ascontiguousarray(
            np.tile(sn_g[None, :], (128, 1)).astype(np.float32)), name="sng")
        snb_d = nc.inline_tensor(np.ascontiguousarray(
            np.tile(sn_b[None, :], (128, 1)).astype(np.float32)), name="snb")

    with tile.TileContext(nc) as tc:
        ctx_pools = []

        def pool(**kw):
            p = tc.tile_pool(**kw)
            ctx_pools.append(p)
            return p.__enter__()

        const = pool(name="const", bufs=1)
        state = pool(name="state", bufs=1)
        cpool = pool(name="cstate", bufs=2)
        hpool = pool(name="hstate", bufs=2)
        gpool = pool(name="gates", bufs=2)
        tmp = pool(name="tmp", bufs=6)
        scr = pool(name="scr", bufs=2)
        tiny = pool(name="tiny", bufs=28)
        stgp = pool(name="stg", bufs=3)
        lhp = pool(name="lhsT", bufs=3)
        zqp = pool(name="zq", bufs=5, space="PSUM")
        aux = pool(name="aux", bufs=3, space="PSUM")

        # ---------------- constants into SBUF ----------------
        W_sb = []
        G_sb = []
        for uc in range(4):
            w_t = const.tile([128, FU], BF16, tag=f"W{uc}")
            nc.sync.dma_start(w_t[:], W_d[uc])
            W_sb.append(w_t)
            g_t = const.tile([128, U], BF16, tag=f"G{uc}")
            nc.sync.dma_start(g_t[:], G_d[uc])
            G_sb.append(g_t)
        wm_sb = const.tile([128, 4], BF16, tag="wm")
        nc.sync.dma_start(wm_sb[:], wm_d[:, :])
        Kt_sb = const.tile([NAUG, FU], BF16, tag="Kt")
        nc.sync.dma_start(Kt_sb[:], Kt_d[:, :])
        M4_sb = const.tile([4, 5], F32, tag="M4")
        nc.sync.dma_start(M4_sb[:], M4_d[:, :])
        Wd_sb = const.tile([128, 4, 4], BF16, tag="Wd")
        nc.sync.dma_start(Wd_sb[:], Wd_d[:, :, :].rearrange("u p d -> p u d"))
        bd_sb = const.tile([128, 4], F32, tag="bd")
        nc.sync.dma_start(bd_sb[:], bd_d[:, :])
        if not sn_trivial:
            sng_sb = const.tile([128, U], F32, tag="sng")
            nc.sync.dma_start(sng_sb[:], sng_d[:, :])
            snb_sb = const.tile([128, U], F32, tag="snb")
            nc.sync.dma_start(snb_sb[:], snb_d[:, :])
        ident = const.tile([128, 128], F32, tag="ident")
        make_identity(nc, ident[:])

        # anchor the ACT table set (sigmoid_and_others holds every func used)
        dummy = tiny.tile([128, 1], F32, tag="tiny")
        nc.vector.memset(dummy[:], 0.0)
        nc.scalar.activation(dummy[:], dummy[:], AF.Sigmoid)

        # ---------------- load x / h0 / c0 ----------------
        x_sb = state.tile([128, NM, T * DIN], F32, tag="x")
        nc.sync.dma_start(
            x_sb[:], x_ext[:, :, :].rearrange("(m p) t d -> p m (t d)", p=128))
        xuT_sb = state.tile([DIN, T, BL], F32, tag="xuT")
        nc.sync.dma_start(xuT_sb[:], xT_ext[:, :, :])
        h_prev = hpool.tile([128, NM, U], F32, tag="h")
        nc.sync.dma_start(
            h_prev[:], h0_ext[:, :].rearrange("(m p) u -> p m u", p=128))
        c_prev = cpool.tile([128, NM, U], F32, tag="c")
        nc.sync.dma_start(
            c_prev[:], c0_ext[:, :].rearrange("(m p) u -> p m u", p=128))

        hT_sb = state.tile([128, 4, BL], BF16, tag="hT")
        out_sb = state.tile([128, NM, t_steps * DIN], F32, tag="out")
        sx_sb = state.tile([128, NM, T], F32, tag="sx")

        def mcols(m):
            return slice(m * 128, (m + 1) * 128)

        def hTf(uc, m):
            return hT_sb[:, uc, mcols(m)]

        # h0 -> hT
        for m in range(NM):
            htp = aux.tile([128, 512], F32, tag="aux")
            for uc in range(4):
                nc.tensor.transpose(
                    htp[:, uc * 128:(uc + 1) * 128],
                    h_prev[:, m, uc * 128:(uc + 1) * 128], ident[:])
            for uc in range(4):
                if uc % 2 == 0:
                    nc.scalar.copy(hT_sb[:, uc, mcols(m)],
                                   htp[:, uc * 128:(uc + 1) * 128])
                else:
                    nc.vector.tensor_copy(hT_sb[:, uc, mcols(m)],
                                          htp[:, uc * 128:(uc + 1) * 128])

        # ------------- x precompute: s_x = rsqrt(var_x + eps) -------------
        for m in range(NM):
            qx_ps = aux.tile([128, 512], F32, tag="aux")
            for t in range(T):
                nc.tensor.matmul(qx_ps[:, t * 5:(t + 1) * 5],
                                 xuT_sb[0:4, t, mcols(m)], M4_sb[:, :],
                                 start=True, stop=True)
            qx_sb = tmp.tile([128, 144], F32, tag="qx")
            nc.scalar.copy(qx_sb[:, 0:T * 5], qx_ps[:, 0:T * 5])
            qv = qx_sb[:, 0:T * 5].rearrange("p (t e) -> p t e", e=5)
            xv = x_sb[:, m, :].rearrange("p (t d) -> p t d", d=DIN)
            pq = tmp.tile([128, T * DIN], F32, tag="pq")
            nc.vector.tensor_mul(
                pq[:].rearrange("p (t d) -> p t d", d=DIN), qv[:, :, 0:4], xv)
            ssx = tiny.tile([128, T], F32, tag="tinyT")
            nc.vector.reduce_sum(ssx[:], pq[:].rearrange(
                "p (t d) -> p t d", d=DIN), axis=X_AX)
            mux = qv[:, :, 4]                                    # [128, 28]
            msq = tiny.tile([128, T], F32, tag="tinyT")
            nc.vector.tensor_mul(msq[:], mux, mux)
            t1m = tiny.tile([128, T], F32, tag="tinyT")
            nc.vector.tensor_scalar(t1m[:], msq[:], -1.0, float(EPS),
                                    OP.mult, OP.add)
            am = tiny.tile([128, T], F32, tag="tinyT")
            nc.vector.tensor_scalar(am[:], ssx[:], 1.0 / FU, None, OP.mult)
            tx = tiny.tile([128, T], F32, tag="tinyT")
            nc.vector.tensor_add(tx[:], am[:], t1m[:])
            sx = _emit_rsqrt(nc, tiny, tx[:], T, 3)              # [128, 28]
            nc.vector.tensor_copy(sx_sb[:, m, :], sx)

        # ---------------- recurrent steps ----------------
        for t in range(t_steps):
            s_list = [None] * NM
            b_list = [None] * NM
            zq_list = [None] * NM
            stats_ps = []
            for m in range(NM):
                mu_ps = aux.tile([128, 512], F32, tag="aux")
                for uc in range(4):
                    nc.tensor.matmul(mu_ps[:, 0:1], hTf(uc, m),
                                     wm_sb[:, uc:uc + 1],
                                     start=(uc == 0), stop=(uc == 3))
                u_ps = aux.tile([128, 512], F32, tag="aux")
                for uc in range(4):
                    nc.tensor.matmul(u_ps[:, :], hT_sb[:, uc, mcols(m)],
                                     G_sb[uc][:, :],
                                     start=(uc == 0), stop=(uc == 3))
                stats_ps.append((mu_ps, u_ps))

            for m in range(NM):
                mu_ps, u_ps = stats_ps[m]
                zqs = [zqp.tile([128, 512], F32, tag="zq", name=f"zq{n}")
                       for n in range(4)]
                zq_list[m] = zqs
                for uc in range(4):
                    for n in range(4):
                        nc.tensor.matmul(zqs[n][:, :],
                                         hT_sb[:, uc, mcols(m)],
                                         W_sb[uc][:, n * 512:(n + 1) * 512],
                                         start=(uc == 0), stop=False)
                # z-LN stats on DVE: sumsq(r) = rowsum((h@G) * h)
                scr_t = scr.tile([128, 512], F32, tag="scr")
                nc.vector.tensor_mul(scr_t[:], u_ps[:, :], h_prev[:, m, :])
                ssr = tiny.tile([128, 1], F32, tag="tiny")
                nc.vector.reduce_sum(ssr[:], scr_t[:], axis=X_AX)
                mum = tiny.tile([128, 1], F32, tag="tiny")
                nc.scalar.copy(mum[:], mu_ps[:, 0:1])
                msq = tiny.tile([128, 1], F32, tag="tiny")
                nc.vector.tensor_mul(msq[:], mum[:], mum[:])
                t1m = tiny.tile([128, 1], F32, tag="tiny")
                nc.vector.tensor_scalar(t1m[:], msq[:], -1.0, float(EPS),
                                        OP.mult, OP.add)
                am = tiny.tile([128, 1], F32, tag="tiny")
                nc.vector.tensor_scalar(am[:], ssr[:], 1.0 / FU, None, OP.mult)
                tz = tiny.tile([128, 1], F32, tag="tiny")
                nc.vector.tensor_add(tz[:], am[:], t1m[:])
                s_m = _emit_rsqrt(nc, tiny, tz[:], 1, 2)         # [128,1]
                s_list[m] = s_m
                sg = tiny.tile([128, 1], F32, tag="tiny")
                nc.vector.tensor_mul(sg[:], tz[:], s_m)          # sqrt(var+eps)
                sxs = tiny.tile([128, 1], F32, tag="tiny")
                nc.vector.tensor_mul(sxs[:], sg[:], sx_sb[:, m, t:t + 1])
                stg = stgp.tile([128, 8], F32, tag="stg")
                nc.vector.tensor_copy(stg[:, 0:4], sxs[:].broadcast_to(
                    [128, 4]))
                if aug_trivial:
                    beta = tiny.tile([128, 1], F32, tag="tiny")
                    nc.vector.tensor_scalar(beta[:], mum[:], s_m, -rng0,
                                            OP.mult, OP.mult)
                    b_list[m] = beta
                else:
                    nc.vector.tensor_scalar(stg[:, 4:5], mum[:], -1.0, None,
                                            OP.mult)
                    nc.vector.tensor_copy(stg[:, 5:6], sg[:])
                smT = aux.tile([128, 512], F32, tag="aux")
                nc.tensor.transpose(smT[0:NAUG, 0:128], stg[:, 0:NAUG],
                                    ident[:])
                lh = lhp.tile([NAUG, 128], BF16, tag="lh")
                if not aug_trivial:
                    nc.scalar.copy(lh[0:NAUG, :], smT[0:NAUG, 0:128])
                nc.vector.tensor_mul(
                    lh[0:4, :], xuT_sb[0:4, t, mcols(m)], smT[0:4, 0:128])
                for n in range(4):
                    nc.tensor.matmul(zqs[n][:, :], lh[:, :],
                                     Kt_sb[:, n * 512:(n + 1) * 512],
                                     start=False, stop=True)

            gates = gpool.tile([128, NM, FU], F32, tag="g")
            for m in range(NM):
                for n in range(4):
                    if aug_trivial:
                        nc.scalar.activation(
                            gates[:, m, n * 512:(n + 1) * 512],
                            zq_list[m][n][:, :], GATE_FUNCS[n],
                            bias=b_list[m], scale=s_list[m])
                    else:
                        nc.scalar.activation(
                            gates[:, m, n * 512:(n + 1) * 512],
                            zq_list[m][n][:, :], GATE_FUNCS[n],
                            scale=s_list[m])

            # ---- c update + c-LN + h ----
            c_cur = cpool.tile([128, NM, U], F32, tag="c")
            h_cur = hpool.tile([128, NM, U], F32, tag="h")
            for m in range(NM):
                t1 = tmp.tile([128, U], F32, tag="t1")
                nc.vector.tensor_mul(t1[:], gates[:, m, 512:1024],
                                     c_prev[:, m, :])
                t2 = tmp.tile([128, U], F32, tag="t2")
                nc.vector.tensor_mul(t2[:], gates[:, m, 0:512],
                                     gates[:, m, 1024:1536])
                cc = tmp.tile([128, U], F32, tag="cc")
                nc.vector.tensor_add(cc[:], t1[:], t2[:])
                st6 = tiny.tile([128, 6], F32, tag="tiny")
                nc.vector.bn_stats(st6[:], cc[:])
                mv = tiny.tile([128, 2], F32, tag="tiny")
                nc.vector.bn_aggr(mv[:], st6[:])
                tcm = tiny.tile([128, 1], F32, tag="tiny")
                nc.vector.tensor_scalar(tcm[:], mv[:, 1:2], float(EPS), None,
                                        OP.add)
                sc = _emit_rsqrt(nc, tiny, tcm[:], 1, 2)
                nmsc = tiny.tile([128, 1], F32, tag="tiny")
                nc.vector.tensor_scalar(nmsc[:], mv[:, 0:1], sc, -1.0,
                                        OP.mult, OP.mult)
                # normalized c (state) on ACT; tanh with fused affine
                if sn_trivial:
                    nc.scalar.activation(c_cur[:, m, :], cc[:], AF.Identity,
                                         bias=nmsc[:], scale=sc)
                    th = tmp.tile([128, U], F32, tag="th")
                    nc.scalar.activation(th[:], cc[:], AF.Tanh,
                                         bias=nmsc[:], scale=sc)
                else:
                    cn0 = tmp.tile([128, U], F32, tag="cn0")
                    nc.scalar.activation(cn0[:], cc[:], AF.Identity,
                                         bias=nmsc[:], scale=sc)
                    cn1 = tmp.tile([128, U], F32, tag="cn1")
                    nc.vector.tensor_mul(cn1[:], cn0[:], sng_sb[:, :])
                    nc.vector.tensor_add(c_cur[:, m, :], cn1[:], snb_sb[:, :])
                    th = tmp.tile([128, U], F32, tag="th")
                    nc.scalar.activation(th[:], c_cur[:, m, :], AF.Tanh)
                nc.vector.tensor_mul(h_cur[:, m, :], gates[:, m, 1536:2048],
                                     th[:])
                # h -> hT for next step / output matmul
                htp = aux.tile([128, 512], F32, tag="aux")
                for uc in range(4):
                    nc.tensor.transpose(htp[:, uc * 128:(uc + 1) * 128],
                                        h_cur[:, m, uc * 128:(uc + 1) * 128],
                                        ident[:])
                for uc in range(4):
                    if uc % 2 == 0:
                        nc.scalar.copy(hT_sb[:, uc, mcols(m)],
                                       htp[:, uc * 128:(uc + 1) * 128])
                    else:
                        nc.vector.tensor_copy(hT_sb[:, uc, mcols(m)],
                                              htp[:, uc * 128:(uc + 1) * 128])
                # out_t = relu(h @ Wd + bd)
                op_ps = aux.tile([128, 512], F32, tag="aux")
                for uc in range(4):
                    nc.tensor.matmul(op_ps[:, 0:4], hTf(uc, m),
                                     Wd_sb[:, uc, :],
                                     start=(uc == 0), stop=(uc == 3))
                if bd_trivial:
                    nc.vector.tensor_scalar(out_sb[:, m, t * 4:(t + 1) * 4],
                                            op_ps[:, 0:4], 0.0, None, OP.max)
                else:
                    ob = tiny.tile([128, 4], F32, tag="tiny")
                    nc.vector.tensor_add(ob[:], op_ps[:, 0:4], bd_sb[:, :])
                    nc.vector.tensor_scalar(out_sb[:, m, t * 4:(t + 1) * 4],
                                            ob[:], 0.0, None, OP.max)
            c_prev = c_cur
            h_prev = h_cur

        nc.sync.dma_start(
            out_ext[:, :, :].rearrange("(m p) t d -> p m (t d)", p=128),
            out_sb[:])

        for p in reversed(ctx_pools):
            p.__exit__(None, None, None)

    nc.compile()
    return nc


_NC_CACHE = {}


def _get_nc(weights):
    key = tuple(hash(w.tobytes()) for w in weights)
    if key not in _NC_CACHE:
        _NC_CACHE.clear()
        _NC_CACHE[key] = _build(weights)
    return _NC_CACHE[key]


def _make_in_maps(inputs):
    f32 = lambda a: np.ascontiguousarray(np.asarray(a, dtype=np.float32))
    x = f32(inputs["x"])
    h0 = f32(inputs["h0"])
    c0 = f32(inputs["c0"])
    return [
        {
            "x": np.ascontiguousarray(x[i * BL:(i + 1) * BL]),
            "xT": np.ascontiguousarray(
                x[i * BL:(i + 1) * BL].transpose(2, 1, 0)),
            "h0": np.ascontiguousarray(h0[i * BL:(i + 1) * BL]),
            "c0": np.ascontiguousarray(c0[i * BL:(i + 1) * BL]),
        }
        for i in range(N_CORES)
    ]


def kernel(**inputs):
    f32 = lambda a: np.ascontiguousarray(np.asarray(a, dtype=np.float32))
    weights = (
        f32(inputs["kernel"]), f32(inputs["rec_kernel"]), f32(inputs["bias"]),
        f32(inputs["kn_g"]), f32(inputs["kn_b"]), f32(inputs["rn_g"]),
        f32(inputs["rn_b"]), f32(inputs["sn_g"]), f32(inputs["sn_b"]),
        f32(inputs["Wd"]), f32(inputs["bd"]),
    )
    nc = _get_nc(weights)
    in_maps = _make_in_maps(inputs)
    res = run_bass_kernel_spmd(nc, in_maps, core_ids=list(range(N_CORES)))
    out = np.concatenate([res.results[i]["out"] for i in range(N_CORES)],
                         axis=0)
    return out.astype(np.float32)


if __name__ == "__main__":
    np.random.seed(0)
    pass
